# revision 16
# baseline (speedup 1.0000x reference)
"""Trainium2 Bass kernel for a causal multi-head attention block.

Problem: y = MHA(x), B=256, N=256 (seq), C=512, H=8 heads, d=64
  Q = x@Wq + bq ; K = x@Wk + bk ; V = x@Wv + bv   (per-head split)
  S = Q K^T ; scaled = (S + causal_mask*-1e5)/sqrt(d) ; P = softmax(scaled)
  y = (P V merged) @ Wo + bo

Sharding: pure data-parallel over batch B across 8 NeuronCores (32 batch
elements per core); weights replicated; no collectives.

v2 design — transposed-score attention (no PE transposes):
 - scores are computed directly as S^T [k, q] = K_h^T-contracted matmul
   (A=kh, B=qh), so the P@V step needs no transpose of P at all:
   attnT[d, q] = sum_k V[k,d] E^T[k,q] with V as the stationary operand.
 - softmax normalization is deferred: E^T = exp((S^T+mask)/sqrt(d)) is fed
   unnormalized into the PV matmul; the stationary operand is [V_h | 1]
   (65 cols), so PSUM row 64 accumulates the softmax denominators for
   free. The at->SBUF evacuation multiplies by 1/rowsum broadcast across
   partitions (gpsimd partition_broadcast), which also casts to bf16.
 - only the two diagonal 128x128 blocks need the additive causal mask;
   the off-diagonal (k-block 0 vs q-block 1) needs none.
 - Wv is interleaved host-side to [C, 8*65] with a zero column per head
   where the ones column of V-hat lives (gpsimd memsets the ones).
 - bv/bo folded on host: softmax rows sum to 1, so V's bias contributes
   attn@(1 bv^T) = bv exactly, hence y += bv@Wo + bo after gathering.

Host-side marshalling inside kernel():
 - x pre-transposed to xT [b, C, N], everything cast to bf16.
"""

import os
import sys

sys.path.insert(0, "/opt/trn_rl_repo")

import numpy as np

import concourse.bass as bass
import concourse.mybir as mybir
import concourse.tile as tile
from concourse import bacc

B, N, C, H, D = 256, 256, 512, 8, 64
NCORES = 8
NB = B // NCORES  # batch elements per core
P = 128
F32 = mybir.dt.float32
MASK_VAL = -100000.0
DV = D + 1  # V-hat cols per head (V | ones)

AF = mybir.ActivationFunctionType

MM_DT = mybir.dt.bfloat16


def _emit(nc: bass.Bass, nb: int):
    xt_in = nc.dram_tensor("xt", [nb, C, N], MM_DT, kind="ExternalInput")
    Wq = nc.dram_tensor("Wq", [C, C], MM_DT, kind="ExternalInput")
    Wk = nc.dram_tensor("Wk", [C, C], MM_DT, kind="ExternalInput")
    Wv = nc.dram_tensor("Wv", [C, C], MM_DT, kind="ExternalInput")
    Wo = nc.dram_tensor("Wo", [C, C], MM_DT, kind="ExternalInput")
    bq = nc.dram_tensor("bq", [C], F32, kind="ExternalInput")
    bk = nc.dram_tensor("bk", [C], F32, kind="ExternalInput")
    y = nc.dram_tensor("y", [nb, N, C], F32, kind="ExternalOutput")

    CB = C // P  # 4 column blocks of 128
    TB = N // P  # 2 row blocks of 128
    VHW = H * DV  # 520
    VHH = VHW // 2  # 260, V-hat GEMM half width

    with (
        tile.TileContext(nc) as tc,
        tc.tile_pool(name="consts", bufs=1) as consts,
        tc.tile_pool(name="io", bufs=3) as io,
        tc.tile_pool(name="work", bufs=int(os.environ.get("WB", "3"))) as work,
        tc.tile_pool(name="heads", bufs=int(os.environ.get("HB", "8"))) as heads,
        tc.tile_pool(name="ps_mm", bufs=int(os.environ.get("PS_MM", "3")), space="PSUM") as ps_mm,
        tc.tile_pool(name="ps_sc", bufs=int(os.environ.get("PS_SC", "3")), space="PSUM") as ps_sc,
        tc.tile_pool(name="ps_at", bufs=int(os.environ.get("PS_AT", "2")), space="PSUM") as ps_at,
    ):
        wq_sb = consts.tile([P, CB, C], MM_DT)
        nc.sync.dma_start(wq_sb, Wq.rearrange("(k p) m -> p k m", p=P))
        wk_sb = consts.tile([P, CB, C], MM_DT)
        nc.sync.dma_start(wk_sb, Wk.rearrange("(k p) m -> p k m", p=P))
        wv_sb = consts.tile([P, CB, C], MM_DT)
        nc.sync.dma_start(wv_sb, Wv.rearrange("(k p) m -> p k m", p=P))
        wo_sb = consts.tile([P, CB, C], MM_DT)
        nc.sync.dma_start(wo_sb, Wo.rearrange("(k p) m -> p k m", p=P))
        bq_sb = consts.tile([P, CB], F32)
        nc.sync.dma_start(bq_sb, bq.rearrange("(m p) -> p m", p=P))
        bk_sb = consts.tile([P, CB], F32)
        nc.sync.dma_start(bk_sb, bk.rearrange("(m p) -> p m", p=P))

        npairs = nb // 2
        NSTEP = 2 * H
        LAG = int(os.environ.get("LAG", "4"))
        HIPRI = int(os.environ.get("HIPRI", "40"))

        def emit_xt(pi):
            xT = work.tile([P, CB, 2 * N], MM_DT, tag="xT")
            for e in range(2):
                nc.sync.dma_start(
                    xT[:, :, e * N : (e + 1) * N],
                    xt_in[pi * 2 + e].rearrange("(cb p) n -> p cb n", p=P),
                )
            return xT

        def build_vq_fillers(xT):
            """Per-pair V-hat + QT/KT projection emitters (16 chunks)."""
            vh0 = work.tile([P, TB, H, P], MM_DT, tag="vh0")
            vh1 = work.tile([P, TB, H, P], MM_DT, tag="vh1")
            vhs = [vh0, vh1]
            qt = work.tile([P, CB, 2 * N], MM_DT, tag="qt")
            kt = work.tile([P, CB, 2 * N], MM_DT, tag="kt")
            fns = []

            def v_chunk(e, t, half):
                eo = e * N
                vh = vhs[e]
                pv = ps_mm.tile([P, 2 * N], F32, tag="mm")
                for k in range(CB):
                    nc.tensor.matmul(
                        pv[:, 0 : 2 * P],
                        xT[:, k, eo + t * P : eo + (t + 1) * P],
                        wv_sb[:, k, half * 2 * P : (half + 1) * 2 * P],
                        start=(k == 0),
                        stop=(k == CB - 1),
                    )
                nc.vector.tensor_copy(
                    vh[:, t, 4 * half : 4 * half + 4, 0:D], pv[:, 0 : 2 * P]
                )
                if t == TB - 1 and half == 1:
                    nc.gpsimd.memset(vh[:, :, :, D:P], 1.0)

            def qk_chunk(mb, which):
                w_sb = wq_sb if which == 0 else wk_sb
                pq = ps_mm.tile([P, 2 * N], F32, tag="mm")
                for k in range(CB):
                    nc.tensor.matmul(
                        pq,
                        w_sb[:, k, mb * P : (mb + 1) * P],
                        xT[:, k, :],
                        start=(k == 0),
                        stop=(k == CB - 1),
                    )
                if which == 0:
                    nc.scalar.activation(
                        qt[:, mb, :], pq, AF.Identity, bias=bq_sb[:, mb : mb + 1]
                    )
                else:
                    nc.vector.tensor_scalar_add(
                        kt[:, mb, :], pq, bk_sb[:, mb : mb + 1]
                    )

            for e in range(2):
                for t in range(TB):
                    for half in range(2):
                        fns.append(lambda e=e, t=t, half=half: v_chunk(e, t, half))
            for mb in range(CB):
                for which in (0, 1):
                    fns.append(lambda mb=mb, which=which: qk_chunk(mb, which))
            return vhs, qt, kt, fns

        # ---- steady-state pipeline over pairs ----
        xT_cur = emit_xt(0)
        vhs_cur, qt_cur, kt_cur, fns0 = build_vq_fillers(xT_cur)
        for f in fns0:
            f()

        for pi in range(npairs):
            if pi + 1 < npairs:
                xT_nxt = emit_xt(pi + 1)
                vhs_nxt, qt_nxt, kt_nxt, fillers = build_vq_fillers(xT_nxt)
            else:
                fillers = []

            at_sb0 = work.tile([P, CB, N], MM_DT, tag="at0")
            at_sb1 = work.tile([P, CB, N], MM_DT, tag="at1")
            at_sbs = [at_sb0, at_sb1]
            ets = {}
            atps = {}
            qt, kt, vhs = qt_cur, kt_cur, vhs_cur

            def emit_scores(e, h):
                eo = e * N
                sub, cb = h % 2, h // 2
                hp = D * sub
                qh = qt[hp : hp + D, cb, eo : eo + N]
                kh = kt[hp : hp + D, cb, eo : eo + N]
                st = ps_sc.tile([P, 3 * P], F32, tag="sc")
                nc.tensor.matmul(
                    st[:, 0 : 2 * P], kh[:, 0:P], qh,
                    start=True, stop=True, skip_group_check=True,
                )
                nc.tensor.matmul(
                    st[:, 2 * P : 3 * P], kh[:, P:N], qh[:, P:N],
                    start=True, stop=True, skip_group_check=True,
                )
                et = heads.tile([P, 3, P], MM_DT, tag="et")
                with tc.high_priority(offset=HIPRI):
                    nc.scalar.activation(et, st, AF.Exp, scale=0.125)
                    # causal: zero strictly-lower triangle of the diag blocks
                    for blk in (et[:, 0, :], et[:, 2, :]):
                        nc.gpsimd.affine_select(
                            out=blk, in_=blk,
                            compare_op=mybir.AluOpType.is_ge,
                            fill=0.0, base=0,
                            pattern=[[1, P]], channel_multiplier=-1,
                        )
                ets[(e, h)] = et

            def emit_pv(e, h):
                sub, hp_i = h % 2, h // 2
                if sub == 0:
                    at_new = ps_at.tile([P, 2 * N], F32, tag="at")
                    atps[(e, hp_i)] = at_new
                at_ps = atps[(e, hp_i)]
                et = ets.pop((e, h))
                vh = vhs[e]
                co = 2 * P * sub
                nc.tensor.matmul(
                    at_ps[:, co : co + 2 * P],
                    vh[:, 0, h, :],
                    et[:, 0:2, :],
                    start=True, stop=False, skip_group_check=True,
                )
                nc.tensor.matmul(
                    at_ps[:, co + P : co + 2 * P],
                    vh[:, 1, h, :],
                    et[:, 2, :],
                    start=False, stop=True, skip_group_check=True,
                )

            def emit_norm(e, hp_i):
                at_ps = atps.pop((e, hp_i))
                at_sb = at_sbs[e]
                rs = heads.tile([D, 2 * N], F32, tag="rs")
                nc.scalar.copy(rs, at_ps[D:P, :])
                rc = heads.tile([D, 2 * N], F32, tag="rc")
                nc.vector.reciprocal_approx_fast(rc, rs)
                nc.vector.tensor_mul(
                    at_sb[0:D, hp_i, :], at_ps[0:D, 0:N], rc[:, 0:N]
                )
                nc.vector.tensor_mul(
                    at_sb[D:P, hp_i, :], at_ps[0:D, N : 2 * N], rc[:, N : 2 * N]
                )

            for v in range(NSTEP + LAG):
                if v < NSTEP:
                    emit_scores(v % 2, v // 2)
                    if fillers:
                        fillers.pop(0)()
                if v >= LAG:
                    u = v - LAG
                    e, h = u % 2, u // 2
                    emit_pv(e, h)
                    if h % 2 == 1:
                        emit_norm(e, h // 2)

            # ---- output projections ----
            for e in range(2):
                at_sb = at_sbs[e]
                for t in range(TB):
                    py = ps_mm.tile([P, C], F32, tag="mm")
                    for k in range(CB):
                        nc.tensor.matmul(
                            py,
                            at_sb[:, k, t * P : (t + 1) * P],
                            wo_sb[:, k, :],
                            start=(k == 0),
                            stop=(k == CB - 1),
                        )
                    y_sb = io.tile([P, C], F32, tag="y")
                    nc.scalar.copy(y_sb, py)
                    nc.sync.dma_start(
                        y[pi * 2 + e].rearrange("(t p) c -> p t c", p=P)[:, t, :],
                        y_sb,
                    )

            if pi + 1 < npairs:
                xT_cur, vhs_cur, qt_cur, kt_cur = xT_nxt, vhs_nxt, qt_nxt, kt_nxt

    return nc


_NC_CACHE: dict = {}


def _build(nb: int = NB) -> bass.Bass:
    key = nb
    if key not in _NC_CACHE:
        nc = bacc.Bacc()
        _emit(nc, nb)
        nc.finalize()
        _NC_CACHE[key] = nc
    return _NC_CACHE[key]


def _run(inputs: dict, nb: int = NB, trace: bool = False):
    """Returns (y_full [8*nb, N, C], BassKernelResults)."""
    from concourse.bass_utils import run_bass_kernel_spmd

    import ml_dtypes

    bf16 = ml_dtypes.bfloat16
    x = np.asarray(inputs["x"], np.float32)[: NCORES * nb]
    xt = np.ascontiguousarray(x.transpose(0, 2, 1)).astype(bf16)
    Wq = np.ascontiguousarray(np.asarray(inputs["Wq"], np.float32).astype(bf16))
    Wk = np.ascontiguousarray(np.asarray(inputs["Wk"], np.float32).astype(bf16))
    Wv = np.ascontiguousarray(np.asarray(inputs["Wv"], np.float32).astype(bf16))
    Wo = np.ascontiguousarray(np.asarray(inputs["Wo"], np.float32).astype(bf16))
    bq = np.ascontiguousarray(np.asarray(inputs["bq"], np.float32))
    bk = np.ascontiguousarray(np.asarray(inputs["bk"], np.float32))
    bv = np.asarray(inputs["bv"], np.float32)
    bo = np.asarray(inputs["bo"], np.float32)

    nc = _build(nb)
    in_maps = [
        {
            "xt": np.ascontiguousarray(xt[c * nb : (c + 1) * nb]),
            "Wq": Wq,
            "Wk": Wk,
            "Wv": Wv,
            "Wo": Wo,
            "bq": bq,
            "bk": bk,
        }
        for c in range(NCORES)
    ]
    res = run_bass_kernel_spmd(nc, in_maps, list(range(NCORES)), trace=trace)
    y = np.concatenate([r["y"] for r in res.results], axis=0)
    # host-side fold of bv/bo (exact: softmax rows sum to 1)
    y = y + (bv @ np.asarray(inputs["Wo"], np.float32) + bo)
    return y, res


def kernel(**inputs) -> np.ndarray:
    y, _ = _run(inputs, nb=NB, trace=False)
    return y.astype(np.float32)
